# revision 7
# baseline (speedup 1.0000x reference)
"""Trainium2 Bass kernel for AttentionConvolution (GNN message passing).

Reference computation (per sample):
    for j in 1, 2:
        mask_j = (adj == j)                       # [N, N]
        d_j    = (mask_j / rowsum(mask_j)) @ hid  # [N, D]
    out = LN(relu(cat(d1, d2) @ W + b) + hid)     # LN over feature dim

Strategy:
  - Data-parallel over batch: 16 samples -> 8 cores, 2 samples each.
  - Host precomputes, per direction, the normalized+transposed mask
      wT_j[m, n] = LAMBDA_M * mask_j[n, m] / (rowsum_j[n] + EPS)
    in fp8e4m3, laid out for DoubleRow matmuls ("k-layout": the
    contraction axis m split into 8 subtiles of 128 partitions).
  - Device: all matmuls run fp8 DoubleRow (2 MACs/cell/cycle):
      main:  catT[k, n] = (hidT-chunk).T @ wT_j    (k = dir*512 + d)
      fc:    fc[n, d']  = (catT-tile).T @ (LAMBDA_W * W)
    PSUM accumulates in fp32. The combined scale 1/(LAMBDA_M*LAMBDA_W)
    is applied inside the ReLU on the Scalar engine. Epilogue (residual
    add, LayerNorm via bn_stats) runs on Vector/Scalar engines in bf16.
"""

import numpy as np
import ml_dtypes

B = 16
N = 1024
D = 512
N_CORES = 8
S = B // N_CORES          # samples per core
NT = N // 128             # n tiles (128 rows each)
KS = 8                    # contraction subtiles (8 x 128 = 1024)
EPS = 1e-13
LN_EPS = 1e-5
LAMBDA_M = 64.0           # scale on normalized masks (keeps fp8 in range)
LAMBDA_W = 8.0            # scale on FC weights (lifts W out of fp8 subnormals)

F8 = ml_dtypes.float8_e4m3
BF16 = ml_dtypes.bfloat16

_CACHED = {}


def _build_nc(has_bias, has_gb):
    import concourse.bacc as bacc
    import concourse.mybir as mybir
    from concourse.tile import TileContext

    f8 = mybir.dt.float8e4
    bf = mybir.dt.bfloat16
    f32 = mybir.dt.float32
    DR = mybir.MatmulPerfMode.DoubleRow
    AF = mybir.ActivationFunctionType
    ADD = mybir.AluOpType.add
    MULT = mybir.AluOpType.mult

    nc = bacc.Bacc()
    wt = nc.declare_dram_parameter("wt", [S * 2, 128, KS, N], f8, isOutput=False)
    h8 = nc.declare_dram_parameter("h8", [S, 128, KS, D], f8, isOutput=False)
    hr = nc.declare_dram_parameter("hr", [S, 128, NT, D], bf, isOutput=False)
    w8 = nc.declare_dram_parameter("w8", [128, KS, D], f8, isOutput=False)
    if has_bias:
        bsc = nc.declare_dram_parameter("bsc", [1, D], f32, isOutput=False)
    if has_gb:
        gB = nc.declare_dram_parameter("gB", [128, D], bf, isOutput=False)
        bB = nc.declare_dram_parameter("bB", [128, D], bf, isOutput=False)
    out = nc.declare_dram_parameter("out", [S, 128, NT, D], bf, isOutput=True)

    with TileContext(nc) as tc:
        with (
            tc.tile_pool(name="pwt", bufs=3) as pwt,        # mask tiles 8KB/part
            tc.tile_pool(name="ph8", bufs=2) as ph8,        # hidT fp8 4KB/part
            tc.tile_pool(name="phr", bufs=2) as phr,        # residual 8KB/part
            tc.tile_pool(name="pconst", bufs=1) as pconst,  # FC weights + misc
            tc.tile_pool(name="pcat", bufs=3) as pcat,      # catT halves 4KB/part
            tc.tile_pool(name="px", bufs=3) as px,          # epilogue tiles
            tc.tile_pool(name="px2", bufs=10) as px2,       # residual sums
            tc.tile_pool(name="pst", bufs=2) as pst,        # stats
            tc.tile_pool(name="pmain", bufs=6, space="PSUM") as pmain,
            tc.tile_pool(name="pfc", bufs=2, space="PSUM") as pfc,
        ):
            w8_sb = pconst.tile([128, KS, D], f8)
            nc.sync.dma_start(out=w8_sb[:], in_=w8[:])
            eps_sb = pconst.tile([128, 1], f32)
            nc.vector.memset(eps_sb[:], LN_EPS)
            if has_bias:
                bsc_sb = pconst.tile([1, D], f32)
                nc.sync.dma_start(out=bsc_sb[:], in_=bsc[:])
                ones_sb = pconst.tile([1, 128], f32)
                nc.vector.memset(ones_sb[:], 1.0)
            if has_gb:
                gB_sb = pconst.tile([128, D], bf)
                nc.sync.dma_start(out=gB_sb[:], in_=gB[:])
                bB_sb = pconst.tile([128, D], bf)
                nc.sync.dma_start(out=bB_sb[:], in_=bB[:])

            for s in range(S):
                # --- load this sample's operands ---
                wt_sb = []
                for j in range(2):
                    t = pwt.tile([128, KS, N], f8, tag="wt")
                    nc.sync.dma_start(out=t[:], in_=wt[s * 2 + j])
                    wt_sb.append(t)
                h8_sb = ph8.tile([128, KS, D], f8)
                nc.sync.dma_start(out=h8_sb[:], in_=h8[s])
                hr_sb = phr.tile([128, NT, D], bf)
                nc.sync.dma_start(out=hr_sb[:], in_=hr[s])

                # per-sample stats: mv[:, 0/1, t] = mean/var per n-tile
                mv = pst.tile([128, 2, NT], f32, tag="mv")
                sd = pst.tile([128, NT], f32, tag="sd")
                rs = pst.tile([128, NT], f32, tag="rs")
                negmu = pst.tile([128, NT], f32, tag="negmu")

                pending = []  # (x2, t_idx) awaiting normalize after stats

                for nh in range(2):  # halves of n (512 columns each)
                    nsl = slice(nh * 512, (nh + 1) * 512)
                    cat_sb = pcat.tile([128, KS, 512], f8, tag="cat")
                    # --- main matmuls: catT[kc] = hidT_chunk.T @ wT_j ---
                    for dc in range(4):
                        lhs_all = h8_sb[:, :, dc * 128:(dc + 1) * 128]
                        pm = [pmain.tile([128, 512], f32, tag="pm",
                                         name=f"pm{j}")
                              for j in range(2)]
                        for mp in range(4):
                            lhsT = lhs_all[:, 2 * mp:2 * mp + 2, :]
                            for j in range(2):
                                nc.tensor.matmul(
                                    pm[j][:],
                                    lhsT,
                                    wt_sb[j][:, 2 * mp:2 * mp + 2, nsl],
                                    start=(mp == 0),
                                    stop=(mp == 3),
                                    perf_mode=DR,
                                )
                        # cast psum -> fp8 catT rows (k-chunk kc = j*4+dc)
                        for j in range(2):
                            nc.any.tensor_copy(cat_sb[:, j * 4 + dc, :],
                                               pm[j][:])

                    # --- FC + epilogue for the 4 n-tiles in this half ---
                    for tloc in range(4):
                        t_idx = nh * 4 + tloc
                        fc = pfc.tile([128, 512], f32, tag="fc")
                        for kp in range(4):
                            nc.tensor.matmul(
                                fc[:],
                                cat_sb[:, 2 * kp:2 * kp + 2,
                                       tloc * 128:(tloc + 1) * 128],
                                w8_sb[:, 2 * kp:2 * kp + 2, :],
                                start=(kp == 0),
                                stop=(kp == 3) and not has_bias,
                                perf_mode=DR,
                            )
                        if has_bias:
                            nc.tensor.matmul(
                                fc[:], ones_sb[:], bsc_sb[:],
                                start=False, stop=True,
                            )
                        # x = relu(fc / (LAMBDA_M * LAMBDA_W))
                        x = px.tile([128, D], bf, tag="x")
                        nc.scalar.activation(
                            x[:], fc[:], AF.Relu,
                            scale=1.0 / (LAMBDA_M * LAMBDA_W),
                        )
                        # x2 = x + residual
                        x2 = px2.tile([128, D], bf, tag="x2")
                        nc.vector.tensor_tensor(
                            out=x2[:], in0=x[:], in1=hr_sb[:, t_idx, :],
                            op=ADD,
                        )
                        # LayerNorm stats
                        st6 = pst.tile([128, 6], f32, tag="st6")
                        nc.vector.bn_stats(st6[:], x2[:])
                        nc.vector.bn_aggr(mv[:, :, t_idx], st6[:])
                        pending.append((x2, t_idx))

                # --- per-sample: rs = 1/sqrt(var+eps); negmu = -mean ---
                nc.scalar.activation(sd[:], mv[:, 1, :], AF.Sqrt, bias=eps_sb[:])
                nc.vector.reciprocal(rs[:], sd[:])
                nc.vector.tensor_scalar_mul(negmu[:], mv[:, 0, :], -1.0)

                # --- finish: y = (x2 - mu) * rs (optionally * gamma + beta) ---
                for (x2, t_idx) in pending:
                    xn = px.tile([128, D], bf, tag="xn")
                    nc.vector.tensor_scalar(
                        out=xn[:], in0=x2[:],
                        scalar1=negmu[:, t_idx:t_idx + 1],
                        scalar2=rs[:, t_idx:t_idx + 1],
                        op0=ADD,
                        op1=MULT,
                    )
                    if has_gb:
                        y2 = px.tile([128, D], bf, tag="y2")
                        nc.vector.tensor_tensor(
                            out=y2[:], in0=xn[:], in1=gB_sb[:], op=MULT)
                        y3 = px.tile([128, D], bf, tag="y3")
                        nc.vector.tensor_tensor(
                            out=y3[:], in0=y2[:], in1=bB_sb[:], op=ADD)
                        nc.sync.dma_start(out=out[s, :, t_idx, :], in_=y3[:])
                    else:
                        nc.sync.dma_start(out=out[s, :, t_idx, :], in_=xn[:])

    nc.compile()
    return nc


def _pack_core(adj_c, hid_c, w8, b, gamma, beta, has_bias, has_gb):
    wt = np.empty((S * 2, 128, KS, N), dtype=F8)
    for s in range(S):
        a = adj_c[s]
        for j in (1, 2):
            m = (a == j)
            cnt = m.sum(axis=1, dtype=np.float32)          # rowsum over m
            scale = LAMBDA_M / (cnt + EPS)                 # [N] (per row n)
            wtj = m.T.astype(np.float32) * scale[None, :]  # [m, n]
            wtj = wtj.reshape(KS, 128, N).transpose(1, 0, 2)
            wt[s * 2 + (j - 1)] = wtj.astype(F8)

    hid_f = hid_c.astype(np.float32, copy=False)
    h8 = hid_f.reshape(S, KS, 128, D).transpose(0, 2, 1, 3)
    h8 = np.ascontiguousarray(h8).astype(F8)
    hr = hid_f.reshape(S, NT, 128, D).transpose(0, 2, 1, 3)
    hr = np.ascontiguousarray(hr).astype(BF16)

    im = {"wt": wt, "h8": h8, "hr": hr, "w8": w8}
    if has_bias:
        im["bsc"] = np.ascontiguousarray(
            (b.astype(np.float32) * LAMBDA_M * LAMBDA_W)[None, :])
    if has_gb:
        im["gB"] = np.ascontiguousarray(
            np.broadcast_to(gamma.astype(np.float32), (128, D))).astype(BF16)
        im["bB"] = np.ascontiguousarray(
            np.broadcast_to(beta.astype(np.float32), (128, D))).astype(BF16)
    return im


def pack_inputs(adj, hid, W, b, gamma, beta):
    has_bias = bool(np.any(b != 0))
    has_gb = bool(np.any(gamma != 1) or np.any(beta != 0))
    w8 = (W.astype(np.float32) * LAMBDA_W).reshape(KS, 128, D)
    w8 = np.ascontiguousarray(w8.transpose(1, 0, 2)).astype(F8)
    in_maps = [
        _pack_core(adj[c * S:(c + 1) * S], hid[c * S:(c + 1) * S],
                   w8, b, gamma, beta, has_bias, has_gb)
        for c in range(N_CORES)
    ]
    return in_maps, has_bias, has_gb


def unpack_output(results):
    outs = []
    for c in range(N_CORES):
        o = np.asarray(results[c]["out"])          # [S, 128, NT, D] bf16
        o = o.transpose(0, 2, 1, 3).reshape(S, N, D)
        outs.append(o)
    return np.concatenate(outs, axis=0).astype(np.float32)


def kernel(adj, hid, W, b, gamma, beta):
    from concourse.bass_utils import run_bass_kernel_spmd

    adj = np.asarray(adj)
    hid = np.asarray(hid)
    W = np.asarray(W)
    b = np.asarray(b)
    gamma = np.asarray(gamma)
    beta = np.asarray(beta)

    in_maps, has_bias, has_gb = pack_inputs(adj, hid, W, b, gamma, beta)

    key = (has_bias, has_gb)
    if key not in _CACHED:
        _CACHED[key] = _build_nc(has_bias, has_gb)
    nc = _CACHED[key]

    res = run_bass_kernel_spmd(nc, in_maps, core_ids=list(range(N_CORES)))
    return unpack_output(res.results)


# revision 11
# speedup vs baseline: 1.2299x; 1.2299x over previous
"""Trainium2 Bass kernel for AttentionConvolution (GNN message passing).

Reference computation (per sample):
    for j in 1, 2:
        mask_j = (adj == j)                       # [N, N]
        d_j    = (mask_j / rowsum(mask_j)) @ hid  # [N, D]
    out = LN(relu(cat(d1, d2) @ W + b) + hid)     # LN over feature dim

Strategy:
  - Data-parallel over batch: 16 samples -> 8 cores, 2 samples each.
  - Host precomputes, per direction, the normalized+transposed mask
      wT_j[m, n] = LAMBDA_M * mask_j[n, m] / (rowsum_j[n] + EPS)
    in fp8e4m3, laid out for DoubleRow matmuls ("k-layout": the
    contraction axis m split into 8 subtiles of 128 partitions).
  - Device: all matmuls run fp8 DoubleRow (2 MACs/cell/cycle):
      main:  catT[k, n] = (hidT-chunk).T @ wT_j    (k = dir*512 + d)
      fc:    fc[n, d']  = (catT-tile).T @ (LAMBDA_W * W)
    PSUM accumulates in fp32. The combined scale 1/(LAMBDA_M*LAMBDA_W)
    is applied inside the ReLU on the Scalar engine. Epilogue (residual
    add, LayerNorm via bn_stats) runs on Vector/Scalar engines in bf16.
"""

import numpy as np
import ml_dtypes

B = 16
N = 1024
D = 512
N_CORES = 8
S = B // N_CORES          # samples per core
NT = N // 128             # n tiles (128 rows each)
KS = 8                    # contraction subtiles (8 x 128 = 1024)
EPS = 1e-13
LN_EPS = 1e-5
LAMBDA_M = 64.0           # scale on normalized masks (keeps fp8 in range)
LAMBDA_W = 8.0            # scale on FC weights (lifts W out of fp8 subnormals)

F8 = ml_dtypes.float8_e4m3
BF16 = ml_dtypes.bfloat16

_CACHED = {}


def _build_nc(has_bias, has_gb):
    import concourse.bacc as bacc
    import concourse.mybir as mybir
    from concourse.tile import TileContext

    f8 = mybir.dt.float8e4
    bf = mybir.dt.bfloat16
    f32 = mybir.dt.float32
    DR = mybir.MatmulPerfMode.DoubleRow
    AF = mybir.ActivationFunctionType
    ADD = mybir.AluOpType.add
    MULT = mybir.AluOpType.mult

    nc = bacc.Bacc()
    # wt chunked per (sample, direction, n-half) so first-needed DMAs finish
    # early (DMA queues drain roughly in issue order)
    wt = nc.declare_dram_parameter("wt", [S, 2, 2, 128, KS, 512], f8,
                                   isOutput=False)
    h8 = nc.declare_dram_parameter("h8", [S, 128, KS, D], f8, isOutput=False)
    hr = nc.declare_dram_parameter("hr", [S, 2, 128, NT // 2, D], bf,
                                   isOutput=False)
    w8 = nc.declare_dram_parameter("w8", [128, KS, D], f8, isOutput=False)
    if has_bias:
        bsc = nc.declare_dram_parameter("bsc", [1, D], f32, isOutput=False)
    if has_gb:
        gB = nc.declare_dram_parameter("gB", [128, D], bf, isOutput=False)
        bB = nc.declare_dram_parameter("bB", [128, D], bf, isOutput=False)
    out = nc.declare_dram_parameter("out", [S, 128, NT, D], bf, isOutput=True)

    with TileContext(nc) as tc:
        with (
            tc.tile_pool(name="pwt", bufs=3) as pwt,        # mask tiles 8KB/part
            tc.tile_pool(name="ph8", bufs=2) as ph8,        # hidT fp8 4KB/part
            tc.tile_pool(name="phr", bufs=2) as phr,        # residual 8KB/part
            tc.tile_pool(name="pconst", bufs=1) as pconst,  # FC weights + misc
            tc.tile_pool(name="pcat", bufs=3) as pcat,      # catT halves 4KB/part
            tc.tile_pool(name="px", bufs=3) as px,          # epilogue tiles
            tc.tile_pool(name="px2", bufs=10) as px2,       # residual sums
            tc.tile_pool(name="pst", bufs=2) as pst,        # stats
            tc.tile_pool(name="pmain", bufs=6, space="PSUM") as pmain,
            tc.tile_pool(name="pfc", bufs=2, space="PSUM") as pfc,
        ):
            w8_sb = pconst.tile([128, KS, D], f8)
            nc.sync.dma_start(out=w8_sb[:], in_=w8[:])
            eps_sb = pconst.tile([128, 1], f32)
            nc.vector.memset(eps_sb[:], LN_EPS)
            if has_bias:
                bsc_sb = pconst.tile([1, D], f32)
                nc.sync.dma_start(out=bsc_sb[:], in_=bsc[:])
                ones_sb = pconst.tile([1, 128], f32)
                nc.vector.memset(ones_sb[:], 1.0)
            if has_gb:
                gB_sb = pconst.tile([128, D], bf)
                nc.sync.dma_start(out=gB_sb[:], in_=gB[:])
                bB_sb = pconst.tile([128, D], bf)
                nc.sync.dma_start(out=bB_sb[:], in_=bB[:])

            for s in range(S):
                # --- loads: issue order == DMA drain order; first-needed
                # first: h8, then wt[j0,nh0], wt[j1,nh0], hr[nh0], ...
                h8_sb = ph8.tile([128, KS, D], f8)
                nc.sync.dma_start(out=h8_sb[:], in_=h8[s])
                wt_sb = {}
                hr_sb = {}
                for nh in range(2):
                    for j in range(2):
                        t = pwt.tile([128, KS, 512], f8, tag="wt",
                                     name=f"wt{j}_{nh}")
                        nc.sync.dma_start(out=t[:], in_=wt[s, j, nh])
                        wt_sb[(j, nh)] = t
                    hrt = pwt.tile([128, NT // 2, D], bf, tag="hr",
                                   name=f"hr{nh}")
                    nc.sync.dma_start(out=hrt[:], in_=hr[s, nh])
                    hr_sb[nh] = hrt

                for nh in range(2):  # halves of n (512 columns each)
                    cat_sb = pcat.tile([128, KS, 512], f8, tag="cat")
                    # --- main matmuls: catT[kc] = hidT_chunk.T @ wT_j ---
                    for j in range(2):
                        for dc in range(4):
                            lhs_all = h8_sb[:, :, dc * 128:(dc + 1) * 128]
                            pm = pmain.tile([128, 512], f32, tag="pm")
                            for mp in range(4):
                                nc.tensor.matmul(
                                    pm[:],
                                    lhs_all[:, 2 * mp:2 * mp + 2, :],
                                    wt_sb[(j, nh)][:, 2 * mp:2 * mp + 2, :],
                                    start=(mp == 0),
                                    stop=(mp == 3),
                                    perf_mode=DR,
                                )
                            # cast psum -> fp8 catT row (k-chunk = j*4+dc)
                            nc.any.tensor_copy(cat_sb[:, j * 4 + dc, :],
                                               pm[:])

                    # per-half stats: mv[:, 0/1, t] = mean/var per n-tile
                    mv = pst.tile([128, 2, 4], f32, tag="mv")
                    sd = pst.tile([128, 4], f32, tag="sd")
                    rs = pst.tile([128, 4], f32, tag="rs")
                    negmu = pst.tile([128, 4], f32, tag="negmu")
                    pending = []

                    # --- FC + epilogue for the 4 n-tiles in this half ---
                    for tloc in range(4):
                        t_idx = nh * 4 + tloc
                        fc = pfc.tile([128, 512], f32, tag="fc")
                        for kp in range(4):
                            nc.tensor.matmul(
                                fc[:],
                                cat_sb[:, 2 * kp:2 * kp + 2,
                                       tloc * 128:(tloc + 1) * 128],
                                w8_sb[:, 2 * kp:2 * kp + 2, :],
                                start=(kp == 0),
                                stop=(kp == 3) and not has_bias,
                                perf_mode=DR,
                            )
                        if has_bias:
                            nc.tensor.matmul(
                                fc[:], ones_sb[:], bsc_sb[:],
                                start=False, stop=True,
                            )
                        # x = relu(fc / (LAMBDA_M * LAMBDA_W))
                        x = px.tile([128, D], bf, tag="x")
                        nc.scalar.activation(
                            x[:], fc[:], AF.Relu,
                            scale=1.0 / (LAMBDA_M * LAMBDA_W),
                        )
                        # x2 = x + residual
                        x2 = px2.tile([128, D], bf, tag="x2")
                        nc.vector.tensor_tensor(
                            out=x2[:], in0=x[:], in1=hr_sb[nh][:, tloc, :],
                            op=ADD,
                        )
                        # LayerNorm stats
                        st6 = pst.tile([128, 6], f32, tag="st6")
                        nc.vector.bn_stats(st6[:], x2[:])
                        nc.vector.bn_aggr(mv[:, :, tloc], st6[:])
                        pending.append((x2, tloc, t_idx))

                    # --- per-half: rs = 1/sqrt(var+eps); negmu = -mean ---
                    nc.scalar.activation(sd[:], mv[:, 1, :], AF.Sqrt,
                                         bias=eps_sb[:])
                    nc.vector.reciprocal(rs[:], sd[:])
                    nc.vector.tensor_scalar_mul(negmu[:], mv[:, 0, :], -1.0)

                    # --- y = (x2 - mu) * rs (optionally * gamma + beta) ---
                    for (x2, tloc, t_idx) in pending:
                        xn = px.tile([128, D], bf, tag="xn")
                        nc.vector.tensor_scalar(
                            out=xn[:], in0=x2[:],
                            scalar1=negmu[:, tloc:tloc + 1],
                            scalar2=rs[:, tloc:tloc + 1],
                            op0=ADD,
                            op1=MULT,
                        )
                        if has_gb:
                            y2 = px.tile([128, D], bf, tag="y2")
                            nc.vector.tensor_tensor(
                                out=y2[:], in0=xn[:], in1=gB_sb[:], op=MULT)
                            y3 = px.tile([128, D], bf, tag="y3")
                            nc.vector.tensor_tensor(
                                out=y3[:], in0=y2[:], in1=bB_sb[:], op=ADD)
                            nc.gpsimd.dma_start(out=out[s, :, t_idx, :],
                                                in_=y3[:])
                        else:
                            nc.gpsimd.dma_start(out=out[s, :, t_idx, :],
                                                in_=xn[:])

    nc.compile()
    return nc


def _pack_core(adj_c, hid_c, w8, b, gamma, beta, has_bias, has_gb):
    wt = np.empty((S, 2, 2, 128, KS, 512), dtype=F8)
    for s in range(S):
        a = adj_c[s]
        for j in (1, 2):
            m = (a == j)
            cnt = m.sum(axis=1, dtype=np.float32)          # rowsum over m
            scale = LAMBDA_M / (cnt + EPS)                 # [N] (per row n)
            wtj = m.T.astype(np.float32) * scale[None, :]  # [m, n]
            wtj = wtj.reshape(KS, 128, N).transpose(1, 0, 2).astype(F8)
            for nh in range(2):
                wt[s, j - 1, nh] = wtj[:, :, nh * 512:(nh + 1) * 512]

    hid_f = hid_c.astype(np.float32, copy=False)
    h8 = hid_f.reshape(S, KS, 128, D).transpose(0, 2, 1, 3)
    h8 = np.ascontiguousarray(h8).astype(F8)
    hr = hid_f.reshape(S, 2, NT // 2, 128, D).transpose(0, 1, 3, 2, 4)
    hr = np.ascontiguousarray(hr).astype(BF16)

    im = {"wt": wt, "h8": h8, "hr": hr, "w8": w8}
    if has_bias:
        im["bsc"] = np.ascontiguousarray(
            (b.astype(np.float32) * LAMBDA_M * LAMBDA_W)[None, :])
    if has_gb:
        im["gB"] = np.ascontiguousarray(
            np.broadcast_to(gamma.astype(np.float32), (128, D))).astype(BF16)
        im["bB"] = np.ascontiguousarray(
            np.broadcast_to(beta.astype(np.float32), (128, D))).astype(BF16)
    return im


def pack_inputs(adj, hid, W, b, gamma, beta):
    has_bias = bool(np.any(b != 0))
    has_gb = bool(np.any(gamma != 1) or np.any(beta != 0))
    w8 = (W.astype(np.float32) * LAMBDA_W).reshape(KS, 128, D)
    w8 = np.ascontiguousarray(w8.transpose(1, 0, 2)).astype(F8)
    in_maps = [
        _pack_core(adj[c * S:(c + 1) * S], hid[c * S:(c + 1) * S],
                   w8, b, gamma, beta, has_bias, has_gb)
        for c in range(N_CORES)
    ]
    return in_maps, has_bias, has_gb


def unpack_output(results):
    outs = []
    for c in range(N_CORES):
        o = np.asarray(results[c]["out"])          # [S, 128, NT, D] bf16
        o = o.transpose(0, 2, 1, 3).reshape(S, N, D)
        outs.append(o)
    return np.concatenate(outs, axis=0).astype(np.float32)


def kernel(adj, hid, W, b, gamma, beta):
    from concourse.bass_utils import run_bass_kernel_spmd

    adj = np.asarray(adj)
    hid = np.asarray(hid)
    W = np.asarray(W)
    b = np.asarray(b)
    gamma = np.asarray(gamma)
    beta = np.asarray(beta)

    in_maps, has_bias, has_gb = pack_inputs(adj, hid, W, b, gamma, beta)

    key = (has_bias, has_gb)
    if key not in _CACHED:
        _CACHED[key] = _build_nc(has_bias, has_gb)
    nc = _CACHED[key]

    res = run_bass_kernel_spmd(nc, in_maps, core_ids=list(range(N_CORES)))
    return unpack_output(res.results)


# revision 21
# speedup vs baseline: 1.3559x; 1.1025x over previous
"""Trainium2 Bass kernel for AttentionConvolution (GNN message passing).

Reference computation (per sample):
    for j in 1, 2:
        mask_j = (adj == j)                       # [N, N]
        d_j    = (mask_j / rowsum(mask_j)) @ hid  # [N, D]
    out = LN(relu(cat(d1, d2) @ W + b) + hid)     # LN over feature dim

Strategy:
  - Data-parallel over batch: 16 samples -> 8 cores, 2 samples each.
  - Host precomputes, per direction, the normalized+transposed mask
      wT_j[m, n] = LAMBDA_M * mask_j[n, m] / (rowsum_j[n] + EPS)
    in fp8e4m3, laid out for DoubleRow matmuls ("k-layout": the
    contraction axis m split into 8 subtiles of 128 partitions).
  - Device: all matmuls run fp8 DoubleRow (2 MACs/cell/cycle):
      main:  catT[k, n] = (hidT-chunk).T @ wT_j    (k = dir*512 + d)
      fc:    fc[n, d']  = (catT-tile).T @ (LAMBDA_W * W)
    PSUM accumulates in fp32. The combined scale 1/(LAMBDA_M*LAMBDA_W)
    is applied inside the ReLU on the Scalar engine. Epilogue (residual
    add, LayerNorm via bn_stats) runs on Vector/Scalar engines in bf16.
"""

import numpy as np
import ml_dtypes

B = 16
N = 1024
D = 512
N_CORES = 8
S = B // N_CORES          # samples per core
NT = N // 128             # n tiles (128 rows each)
KS = 8                    # contraction subtiles (8 x 128 = 1024)
EPS = 1e-13
LN_EPS = 1e-5
LAMBDA_M = 64.0           # scale on normalized masks (keeps fp8 in range)
LAMBDA_W = 8.0            # scale on FC weights (lifts W out of fp8 subnormals)

F8 = ml_dtypes.float8_e4m3
BF16 = ml_dtypes.bfloat16

_CACHED = {}


def _build_nc(has_bias, has_gb):
    import concourse.bacc as bacc
    import concourse.mybir as mybir
    from concourse.tile import TileContext

    f8 = mybir.dt.float8e4
    bf = mybir.dt.bfloat16
    f32 = mybir.dt.float32
    DR = mybir.MatmulPerfMode.DoubleRow
    AF = mybir.ActivationFunctionType
    ADD = mybir.AluOpType.add
    SUB = mybir.AluOpType.subtract
    MULT = mybir.AluOpType.mult

    nc = bacc.Bacc()
    # wt chunked per (sample, direction, n-half) so first-needed DMAs finish
    # early (DMA queues drain roughly in issue order)
    wt = nc.declare_dram_parameter("wt", [S, 2, 2, 128, KS, 512], f8,
                                   isOutput=False)
    h8 = nc.declare_dram_parameter("h8", [S, 128, KS, D], f8, isOutput=False)
    hr = nc.declare_dram_parameter("hr", [S, 2, 128, NT // 2, D], bf,
                                   isOutput=False)
    w8 = nc.declare_dram_parameter("w8", [128, KS, D], f8, isOutput=False)
    if has_bias:
        bsc = nc.declare_dram_parameter("bsc", [1, D], f32, isOutput=False)
    if has_gb:
        gB = nc.declare_dram_parameter("gB", [128, D], bf, isOutput=False)
        bB = nc.declare_dram_parameter("bB", [128, D], bf, isOutput=False)
    out = nc.declare_dram_parameter("out", [S, 128, NT, D], bf, isOutput=True)

    with TileContext(nc) as tc:
        with (
            tc.tile_pool(name="pwt", bufs=3) as pwt,        # mask tiles 8KB/part
            tc.tile_pool(name="ph8", bufs=2) as ph8,        # hidT fp8 4KB/part
            tc.tile_pool(name="phr", bufs=2) as phr,        # residual 8KB/part
            tc.tile_pool(name="pconst", bufs=1) as pconst,  # FC weights + misc
            tc.tile_pool(name="pcat", bufs=3) as pcat,      # catT halves 4KB/part
            tc.tile_pool(name="px", bufs=3) as px,          # epilogue tiles
            tc.tile_pool(name="px2", bufs=10) as px2,       # residual sums
            tc.tile_pool(name="pst", bufs=2) as pst,        # stats
            tc.tile_pool(name="pmain", bufs=6, space="PSUM") as pmain,
            tc.tile_pool(name="pfc", bufs=2, space="PSUM") as pfc,
        ):
            w8_sb = pconst.tile([128, KS, D], f8)
            nc.sync.dma_start(out=w8_sb[:], in_=w8[:])
            eps_sb = pconst.tile([128, 1], f32)
            nc.vector.memset(eps_sb[:], LN_EPS)
            if has_bias:
                bsc_sb = pconst.tile([1, D], f32)
                nc.sync.dma_start(out=bsc_sb[:], in_=bsc[:])
                ones_sb = pconst.tile([1, 128], f32)
                nc.vector.memset(ones_sb[:], 1.0)
            if has_gb:
                gB_sb = pconst.tile([128, D], bf)
                nc.sync.dma_start(out=gB_sb[:], in_=gB[:])
                bB_sb = pconst.tile([128, D], bf)
                nc.sync.dma_start(out=bB_sb[:], in_=bB[:])

            for s in range(S):
                # --- loads: issue order == DMA drain order; first-needed
                # first: h8, then wt[j0,nh0], wt[j1,nh0], hr[nh0], ...
                h8_sb = ph8.tile([128, KS, D], f8)
                nc.sync.dma_start(out=h8_sb[:], in_=h8[s])
                wt_sb = {}
                hr_sb = {}
                for nh in range(2):
                    for j in range(2):
                        t = pwt.tile([128, KS, 512], f8, tag="wt",
                                     name=f"wt{j}_{nh}")
                        nc.sync.dma_start(out=t[:], in_=wt[s, j, nh])
                        wt_sb[(j, nh)] = t
                    hrt = pwt.tile([128, NT // 2, D], bf, tag="hr",
                                   name=f"hr{nh}")
                    nc.sync.dma_start(out=hrt[:], in_=hr[s, nh])
                    hr_sb[nh] = hrt

                for nh in range(2):  # halves of n (512 columns each)
                    cat_sb = pcat.tile([128, KS, 512], f8, tag="cat")
                    # --- main matmuls: catT[kc] = hidT_chunk.T @ wT_j ---
                    for j in range(2):
                        for dc in range(4):
                            lhs_all = h8_sb[:, :, dc * 128:(dc + 1) * 128]
                            pm = pmain.tile([128, 512], f32, tag="pm")
                            for mp in range(4):
                                nc.tensor.matmul(
                                    pm[:],
                                    lhs_all[:, 2 * mp:2 * mp + 2, :],
                                    wt_sb[(j, nh)][:, 2 * mp:2 * mp + 2, :],
                                    start=(mp == 0),
                                    stop=(mp == 3),
                                    perf_mode=DR,
                                )
                            # cast psum -> fp8 catT row (k-chunk = j*4+dc)
                            nc.any.tensor_copy(cat_sb[:, j * 4 + dc, :],
                                               pm[:])

                    # per-half stats: mv[:, 0/1, t] = mean/var per n-tile
                    mv = pst.tile([128, 2, 4], f32, tag="mv")
                    sd = pst.tile([128, 4], f32, tag="sd")
                    rs = pst.tile([128, 4], f32, tag="rs")
                    pending = []

                    # --- FC + epilogue for the 4 n-tiles in this half ---
                    for tloc in range(4):
                        t_idx = nh * 4 + tloc
                        fc = pfc.tile([128, 512], f32, tag="fc")
                        for kp in range(4):
                            nc.tensor.matmul(
                                fc[:],
                                cat_sb[:, 2 * kp:2 * kp + 2,
                                       tloc * 128:(tloc + 1) * 128],
                                w8_sb[:, 2 * kp:2 * kp + 2, :],
                                start=(kp == 0),
                                stop=(kp == 3) and not has_bias,
                                perf_mode=DR,
                            )
                        if has_bias:
                            nc.tensor.matmul(
                                fc[:], ones_sb[:], bsc_sb[:],
                                start=False, stop=True,
                            )
                        # x = relu(fc / (LAMBDA_M * LAMBDA_W))
                        x = px.tile([128, D], bf, tag="x")
                        nc.scalar.activation(
                            x[:], fc[:], AF.Relu,
                            scale=1.0 / (LAMBDA_M * LAMBDA_W),
                        )
                        # x2 = x + residual
                        x2 = px2.tile([128, D], bf, tag="x2")
                        nc.vector.tensor_tensor(
                            out=x2[:], in0=x[:], in1=hr_sb[nh][:, tloc, :],
                            op=ADD,
                        )
                        # LayerNorm stats
                        st6 = pst.tile([128, 6], f32, tag="st6")
                        nc.vector.bn_stats(st6[:], x2[:])
                        nc.vector.bn_aggr(mv[:, :, tloc], st6[:])
                        pending.append((x2, tloc, t_idx))

                    # --- per-half: rs = 1/sqrt(var+eps) ---
                    nc.scalar.activation(sd[:], mv[:, 1, :], AF.Sqrt,
                                         bias=eps_sb[:])
                    nc.vector.reciprocal(rs[:], sd[:])

                    # --- y = (x2 - mu) * rs (optionally * gamma + beta) ---
                    for (x2, tloc, t_idx) in pending:
                        xn = px.tile([128, D], bf, tag="xn")
                        nc.vector.tensor_scalar(
                            out=xn[:], in0=x2[:],
                            scalar1=mv[:, 0, tloc:tloc + 1],
                            scalar2=rs[:, tloc:tloc + 1],
                            op0=SUB,
                            op1=MULT,
                        )
                        if has_gb:
                            y2 = px.tile([128, D], bf, tag="y2")
                            nc.vector.tensor_tensor(
                                out=y2[:], in0=xn[:], in1=gB_sb[:], op=MULT)
                            y3 = px.tile([128, D], bf, tag="y3")
                            nc.vector.tensor_tensor(
                                out=y3[:], in0=y2[:], in1=bB_sb[:], op=ADD)
                            nc.gpsimd.dma_start(out=out[s, :, t_idx, :],
                                                in_=y3[:])
                        else:
                            nc.gpsimd.dma_start(out=out[s, :, t_idx, :],
                                                in_=xn[:])

    nc.compile()
    return nc


def _pack_core(adj_c, hid_c, w8, b, gamma, beta, has_bias, has_gb):
    wt = np.empty((S, 2, 2, 128, KS, 512), dtype=F8)
    for s in range(S):
        a = adj_c[s]
        for j in (1, 2):
            m = (a == j)
            cnt = m.sum(axis=1, dtype=np.float32)          # rowsum over m
            scale = LAMBDA_M / (cnt + EPS)                 # [N] (per row n)
            wtj = m.T.astype(np.float32) * scale[None, :]  # [m, n]
            wtj = wtj.reshape(KS, 128, N).transpose(1, 0, 2).astype(F8)
            for nh in range(2):
                wt[s, j - 1, nh] = wtj[:, :, nh * 512:(nh + 1) * 512]

    hid_f = hid_c.astype(np.float32, copy=False)
    h8 = hid_f.reshape(S, KS, 128, D).transpose(0, 2, 1, 3)
    h8 = np.ascontiguousarray(h8).astype(F8)
    hr = hid_f.reshape(S, 2, NT // 2, 128, D).transpose(0, 1, 3, 2, 4)
    hr = np.ascontiguousarray(hr).astype(BF16)

    im = {"wt": wt, "h8": h8, "hr": hr, "w8": w8}
    if has_bias:
        im["bsc"] = np.ascontiguousarray(
            (b.astype(np.float32) * LAMBDA_M * LAMBDA_W)[None, :])
    if has_gb:
        im["gB"] = np.ascontiguousarray(
            np.broadcast_to(gamma.astype(np.float32), (128, D))).astype(BF16)
        im["bB"] = np.ascontiguousarray(
            np.broadcast_to(beta.astype(np.float32), (128, D))).astype(BF16)
    return im


def pack_inputs(adj, hid, W, b, gamma, beta):
    has_bias = bool(np.any(b != 0))
    has_gb = bool(np.any(gamma != 1) or np.any(beta != 0))
    w8 = (W.astype(np.float32) * LAMBDA_W).reshape(KS, 128, D)
    w8 = np.ascontiguousarray(w8.transpose(1, 0, 2)).astype(F8)
    in_maps = [
        _pack_core(adj[c * S:(c + 1) * S], hid[c * S:(c + 1) * S],
                   w8, b, gamma, beta, has_bias, has_gb)
        for c in range(N_CORES)
    ]
    return in_maps, has_bias, has_gb


def unpack_output(results):
    outs = []
    for c in range(N_CORES):
        o = np.asarray(results[c]["out"])          # [S, 128, NT, D] bf16
        o = o.transpose(0, 2, 1, 3).reshape(S, N, D)
        outs.append(o)
    return np.concatenate(outs, axis=0).astype(np.float32)


def kernel(adj, hid, W, b, gamma, beta):
    from concourse.bass_utils import run_bass_kernel_spmd

    adj = np.asarray(adj)
    hid = np.asarray(hid)
    W = np.asarray(W)
    b = np.asarray(b)
    gamma = np.asarray(gamma)
    beta = np.asarray(beta)

    in_maps, has_bias, has_gb = pack_inputs(adj, hid, W, b, gamma, beta)

    key = (has_bias, has_gb)
    if key not in _CACHED:
        _CACHED[key] = _build_nc(has_bias, has_gb)
    nc = _CACHED[key]

    res = run_bass_kernel_spmd(nc, in_maps, core_ids=list(range(N_CORES)))
    return unpack_output(res.results)
